# revision 33
# baseline (speedup 1.0000x reference)
"""Causal self-attention (B=1, T=2048, E=2048, 16 heads, RoPE) on 8 TRN2 NeuronCores.

Strategy: tensor-parallel over heads (2 heads/core). Each core computes
QKV for its heads, RoPE, causal softmax attention, and a PARTIAL output
projection over its 256 contraction columns of w_proj. The host sums the
8 partial [T, E] outputs (no on-device collectives).

All matmul operands are bfloat16 (fp32 PSUM accumulation): same PE rate
as f32r at these tile sizes, but half the DMA bytes, half the LDWEIGHTS
cycles, and 2-4x DVE throughput. End-to-end max-rel error ~4e-3.

Fused single-pass schedule per 512-query chunk j:
  QKV(j) matmuls + RoPE  |  scores S^T(j,h,k) -> exp -> P  |  PV + denom
  with proj(j-1) matmuls as PE filler while ACT drains exps.
Scores are computed transposed (S^T[kt, qt] = kT.T @ qT) so P^T feeds the
V matmul directly and out^T [d, qt] is exactly the lhsT the projection
needs. Causal masking: only lower-triangle kt-tiles computed; diagonal
tiles get multiplicative 0/1 masks after exp. Normalization deferred:
out^T = V^T P^T scaled by broadcast(1/rowsum), where the row sums come
from an elementwise tree-accumulation of the P tiles (DVE + gpsimd)
followed by a single ones-vector matmul, and 1/x = exp(-ln(x)) on ACT
(the DVE reciprocal instruction costs 3.3us).
"""
import sys

for _p in ("/opt/trn_rl_repo",):
    if _p not in sys.path:
        sys.path.append(_p)

import numpy as np

B, T, E = 1, 2048, 2048
H, D = 16, 128
N_CORES = 8
HPC = H // N_CORES          # heads per core
CL = HPC * D                # contraction columns per core (256)
QC = 512                    # qt chunk (PSUM bank width in fp32)
BASE = 10000.0

_CACHE: dict = {}


# ---------------------------------------------------------------- device build
def _build_nc(t=T, debug_stop=None):
    import concourse.tile as tile
    from concourse import bacc, mybir
    from contextlib import ExitStack

    f32 = mybir.dt.float32
    f32r = mybir.dt.float32r
    bf16 = mybir.dt.bfloat16
    nj = t // QC            # qt chunks
    ntt = t // 128          # t tiles
    nct = E // 128          # contraction tiles
    nhalf = nct // 2

    nc = bacc.Bacc("TRN2", target_bir_lowering=False, debug=False,
                   enable_asserts=False, num_devices=N_CORES)
    # x pre-transposed, per-partition contiguous: [j, half, 128, 8*QC]
    xT_d = nc.dram_tensor("xT", [t // QC, 2, 128, nhalf * QC], bf16,
                          kind="ExternalInput").ap()
    wqkvT_d = nc.dram_tensor("wqkvT", [E, 6 * 128], bf16,
                             kind="ExternalInput").ap()
    wprojT_d = nc.dram_tensor("wprojT", [CL, E], bf16, kind="ExternalInput").ap()
    cos2_d = nc.dram_tensor("cos2", [128, t], bf16, kind="ExternalInput").ap()
    sin2_d = nc.dram_tensor("sin2", [128, t], bf16, kind="ExternalInput").ap()
    masks_d = nc.dram_tensor("masks", [4, 128, QC], bf16,
                             kind="ExternalInput").ap()
    out_d = nc.dram_tensor("out", [t, E], bf16, kind="ExternalOutput").ap()

    Exp = mybir.ActivationFunctionType.Exp
    Ln = mybir.ActivationFunctionType.Ln

    with tile.TileContext(nc) as tc:
        with ExitStack() as per:  # persistent pools
            const = per.enter_context(tc.tile_pool(name="const", bufs=1))
            wpp = per.enter_context(tc.tile_pool(name="wpp", bufs=1))
            wqp = per.enter_context(tc.tile_pool(name="wqp", bufs=1))
            qkp = per.enter_context(tc.tile_pool(name="qkp", bufs=1))
            vp = per.enter_context(tc.tile_pool(name="vp", bufs=1))
            atp = per.enter_context(tc.tile_pool(name="atp", bufs=1))
            cstp = per.enter_context(tc.tile_pool(name="cstp", bufs=1))
            mkp = per.enter_context(tc.tile_pool(name="mkp", bufs=1))
            xtr = per.enter_context(tc.tile_pool(name="xtr", bufs=5))
            rtmp = per.enter_context(tc.tile_pool(name="rtmp", bufs=2))
            ptp = per.enter_context(tc.tile_pool(name="ptp", bufs=32))
            ctmp = per.enter_context(tc.tile_pool(name="ctmp", bufs=4))
            outp = per.enter_context(tc.tile_pool(name="outp", bufs=3))
            ps = per.enter_context(tc.tile_pool(name="ps", bufs=1, space="PSUM"))

            # ones vectors
            ones_row_f = const.tile([1, 128], f32)
            nc.vector.memset(ones_row_f[:], 1.0)
            ones_row = const.tile([1, 128], f32r)
            nc.vector.tensor_copy(ones_row[:], ones_row_f[:])
            ones_col_f = const.tile([128, 1], f32)
            nc.vector.memset(ones_col_f[:], 1.0)
            ones_col = const.tile([128, 1], bf16)
            nc.vector.tensor_copy(ones_col[:], ones_col_f[:])

            # persistent activations (bf16)
            qk_sb = [qkp.tile([128, t], bf16, tag=f"qk{f}", name=f"qk{f}")
                     for f in range(4)]
            v_sb = [vp.tile([128, 2 * D], bf16, tag=f"v{g}", name=f"v{g}")
                    for g in range(ntt)]
            attn_sb = [atp.tile([128, t], bf16, tag=f"at{h}", name=f"at{h}")
                       for h in range(HPC)]

            # ---------- prologue DMAs ----------
            # The vector sequencer boots earliest (~4us vs ~8-9 for
            # sync/scalar/gpsimd), so it carries the critical-path start:
            # first weight tiles, then chunk-0 x. Remaining weights on sync,
            # tables/masks on gpsimd, proj weights on scalar.
            WC = 6 * 128
            wq_all = wqp.tile([128, nct * WC], bf16)
            wsrc = wqkvT_d.rearrange("(c p) f -> p c f", c=nct)

            def dma_w(eng, c0, c1):
                dst = wq_all[:, c0 * WC:c1 * WC].rearrange(
                    "p (c f) -> p c f", c=c1 - c0)
                eng.dma_start(out=dst, in_=wsrc[:, c0:c1, :])

            def wq(c, lo, hi):
                return wq_all[:, c * WC + lo * 128:c * WC + hi * 128]

            cos2_sb = cstp.tile([128, t], bf16)
            sin2_sb = cstp.tile([128, t], bf16)
            mask_sb = mkp.tile([128, 4 * QC], bf16)
            wp_sb = []
            for hh in range(HPC):
                wp_sb.append(wpp.tile([128, E], bf16, tag=f"wp{hh}",
                                      name=f"wp{hh}"))

            def dma_x(j, eng=None, split=False):
                eng = eng or nc.sync
                xts = []
                for q in range(2):
                    xh = xtr.tile([128, nhalf * QC], bf16, tag="xt",
                                  name=f"xt{j}_{q}")
                    if split and q == 0:
                        eng.dma_start(out=xh[:, 0:4 * QC],
                                      in_=xT_d[j, q][:, 0:4 * QC])
                        eng.dma_start(out=xh[:, 4 * QC:],
                                      in_=xT_d[j, q][:, 4 * QC:])
                    else:
                        eng.dma_start(out=xh[:], in_=xT_d[j, q])
                    for cc in range(nhalf):
                        xts.append(xh[:, cc * QC:(cc + 1) * QC])
                return xts

            dma_w(nc.gpsimd, 0, 2)
            dma_w(nc.scalar, 2, 9)
            dma_w(nc.sync, 9, 16)
            x_tiles = {0: dma_x(0, eng=nc.gpsimd, split=True)}
            x_tiles[1] = dma_x(1, eng=nc.sync)
            for hh in range(HPC):
                nc.scalar.dma_start(out=wp_sb[hh][:],
                                    in_=wprojT_d[hh * 128:(hh + 1) * 128, :])
            nc.scalar.dma_start(out=cos2_sb[:], in_=cos2_d[:])
            nc.gpsimd.dma_start(out=sin2_sb[:], in_=sin2_d[:])
            nc.gpsimd.dma_start(
                out=mask_sb[:].rearrange("p (o b) -> p o b", o=4),
                in_=masks_d.rearrange("o p b -> p o b"))

            # ---------- fused chunk loop ----------
            def emit_rope(j, f, pq):
                """RoPE on DVE: out = pq*cos2 + swap_halves(pq)*sin2,
                with sin2 = [-sin; sin]; writes qk_sb[f] in bf16."""
                jsl = slice(j * QC, (j + 1) * QC)
                tA = rtmp.tile([128, QC], bf16, tag="tA", name=f"tA{j}_{f}")
                nc.vector.tensor_mul(tA[:], pq[:], cos2_sb[:, jsl])
                tB = rtmp.tile([128, QC], bf16, tag="tB", name=f"tB{j}_{f}")
                nc.vector.tensor_mul(tB[0:64, :], pq[64:128, :],
                                     sin2_sb[0:64, jsl])
                nc.vector.tensor_mul(tB[64:128, :], pq[0:64, :],
                                     sin2_sb[64:128, jsl])
                nc.vector.tensor_add(qk_sb[f][:, jsl], tA[:], tB[:])

            def emit_qkv_f(j, f, xts):
                """q/k tile f for chunk j: 16-matmul group + RoPE -> qk_sb."""
                pq = ps.tile([128, QC], f32, tag="q", bufs=3, name=f"pq{j}_{f}")
                for c in range(nct):
                    nc.tensor.matmul(pq[:], wq(c, f, f + 1), xts[c],
                                     start=(c == 0), stop=(c == nct - 1))
                emit_rope(j, f, pq)

            def emit_v(j, tt, xts):
                pv = ps.tile([128, 2 * D], f32, tag="q", bufs=3,
                             name=f"pv{j}_{tt}")
                for c in range(nct):
                    nc.tensor.matmul(pv[:], xts[c][:, tt * 128:(tt + 1) * 128],
                                     wq(c, 4, 6),
                                     start=(c == 0), stop=(c == nct - 1))
                if tt % 2 == 0:
                    nc.scalar.copy(v_sb[j * 4 + tt][:], pv[:])
                else:
                    nc.vector.tensor_copy(v_sb[j * 4 + tt][:], pv[:])

            def emit_score(j, h, k, pts):
                """S^T tile (j,h,k): matmul -> exp (-> mask) -> P tile."""
                jsl = slice(j * QC, (j + 1) * QC)
                stp = ps.tile([128, QC], f32, tag="s", bufs=3,
                              name=f"st{j}_{h}_{k}")
                nc.tensor.matmul(stp[:], qk_sb[2 + h][:, k * 128:(k + 1) * 128],
                                 qk_sb[h][:, jsl], start=True, stop=True)
                pt = ptp.tile([128, QC], bf16, tag="pt", name=f"pt{j}_{h}_{k}")
                o = k - 4 * j
                if o >= 0:
                    e = ctmp.tile([128, QC], bf16, tag="e", name=f"e{j}_{h}_{k}")
                    nc.scalar.activation(e[:], stp[:], Exp)
                    nc.vector.tensor_mul(pt[:], e[:],
                                         mask_sb[:, o * QC:(o + 1) * QC])
                else:
                    nc.scalar.activation(pt[:], stp[:], Exp)
                pts[(h, k)] = pt

            def emit_pv(j, h, nkt, pts):
                po = ps.tile([128, QC], f32, tag="o", bufs=2, name=f"po{j}_{h}")
                for k in range(nkt):
                    nc.tensor.matmul(po[:], v_sb[k][:, h * D:(h + 1) * D],
                                     pts[(h, k)][:], start=(k == 0),
                                     stop=(k == nkt - 1))
                return po

            def emit_den(j, h, nkt, pts):
                """denominator: ones-matmul accumulation over the P tiles,
                then a one-instruction DVE approximate reciprocal (~18 bits;
                the exact DVE reciprocal costs 3.3us, ACT Ln/Exp thrashes
                activation tables)."""
                ssp = ps.tile([1, QC], f32, tag="s", bufs=3,
                              name=f"ssp{j}_{h}")
                for k in range(nkt):
                    nc.tensor.matmul(ssp[:], ones_col[:], pts[(h, k)][:],
                                     start=(k == 0), stop=(k == nkt - 1))
                rcp = ctmp.tile([1, QC], f32, tag="rcp", bufs=2,
                                name=f"rcp{j}_{h}")
                nc.vector.reciprocal_approx_fast(out=rcp[:], in_=ssp[:])
                inv = ctmp.tile([1, QC], f32r, tag="inv", bufs=4,
                                name=f"inv{j}_{h}")
                with nc.allow_low_precision(reason="f32r softmax denominators"):
                    nc.vector.tensor_copy(inv[:], rcp[:])
                return inv

            def emit_bc(j, h, inv):
                """broadcast 1/sum across partitions via rank-1 matmul."""
                bc = ps.tile([128, QC], f32, tag="q", bufs=3, name=f"bc{j}_{h}")
                nc.tensor.matmul(bc[:], ones_row[:], inv[:], start=True,
                                 stop=True)
                bcs = ctmp.tile([128, QC], f32, tag="bcs", bufs=2,
                                name=f"bcs{j}_{h}")
                nc.scalar.copy(bcs[:], bc[:])
                return bcs

            def emit_normmul(j, h, po, bcs):
                jsl = slice(j * QC, (j + 1) * QC)
                nc.vector.tensor_mul(attn_sb[h][:, jsl], po[:], bcs[:])

            def emit_proj(jj, tts):
                """projection + output DMA for t-tiles tts of chunk jj."""
                for tt in tts:
                    ob = outp.tile([128, E], bf16, tag="ob", name=f"obD{tt}")
                    for oc in range(E // 512):
                        pp = ps.tile([128, 512], f32, tag="q", bufs=3,
                                     name=f"pp{tt}_{oc}")
                        for h in range(HPC):
                            nc.tensor.matmul(
                                pp[:], attn_sb[h][:, tt * 128:(tt + 1) * 128],
                                wp_sb[h][:, oc * 512:(oc + 1) * 512],
                                start=(h == 0), stop=(h == HPC - 1))
                        if oc % 2 == 0:
                            nc.vector.tensor_copy(
                                ob[:, oc * 512:(oc + 1) * 512], pp[:])
                        else:
                            nc.scalar.copy(
                                ob[:, oc * 512:(oc + 1) * 512], pp[:])
                    eng = nc.sync if tt % 2 == 0 else nc.scalar
                    eng.dma_start(out=out_d[tt * 128:(tt + 1) * 128, :],
                                  in_=ob[:])

            for j in range(nj):
                xts = x_tiles.pop(j)
                if j + 2 < nj:
                    x_tiles[j + 2] = dma_x(j + 2)
                nkt = 4 * (j + 1)
                pts: dict = {}

                if j == 0:
                    # c-major first chunk: 8 concurrent PSUM accumulation
                    # groups so compute paces the weight/x DMA arrival
                    # instead of waiting for the full contraction's worth.
                    qt = [ps.tile([128, QC], f32, tag="q" if f < 2 else "s",
                                  bufs=3, name=f"pq0_{f}") for f in range(4)]
                    vtags = ("q", "s", "o", "o")
                    vt = [ps.tile([128, 2 * D], f32, tag=vtags[tt],
                                  bufs=2 if vtags[tt] == "o" else 3,
                                  name=f"pv0_{tt}") for tt in range(4)]
                    for c in range(nct):
                        for f in range(4):
                            nc.tensor.matmul(qt[f][:], wq(c, f, f + 1), xts[c],
                                             start=(c == 0),
                                             stop=(c == nct - 1))
                        for tt in range(4):
                            nc.tensor.matmul(
                                vt[tt][:], xts[c][:, tt * 128:(tt + 1) * 128],
                                wq(c, 4, 6), start=(c == 0),
                                stop=(c == nct - 1))
                    for f in range(4):
                        emit_rope(0, f, qt[f])
                    for tt in range(4):
                        if tt % 2 == 0:
                            nc.scalar.copy(v_sb[tt][:], vt[tt][:])
                        else:
                            nc.vector.tensor_copy(v_sb[tt][:], vt[tt][:])
                    for k in range(nkt):
                        emit_score(j, 0, k, pts)
                    for k in range(nkt):
                        emit_score(j, 1, k, pts)
                else:
                    # QKV interleaved with h=0 scores (old k-tiles need only
                    # q(j,h0))
                    emit_qkv_f(j, 0, xts)
                    old = list(range(4 * j))   # k-tiles from earlier chunks
                    emit_qkv_f(j, 1, xts)
                    for k in old[0:4]:
                        emit_score(j, 0, k, pts)
                    emit_qkv_f(j, 2, xts)
                    for k in old[4:8]:
                        emit_score(j, 0, k, pts)
                    emit_qkv_f(j, 3, xts)
                    for k in old[8:12]:
                        emit_score(j, 0, k, pts)
                    # diagonal tiles need k(j) (f=2,3 above)
                    for k in range(4 * j, nkt):
                        emit_score(j, 0, k, pts)
                    emit_v(j, 0, xts)
                    emit_v(j, 1, xts)
                    vq = [2, 3]  # remaining v tiles, interleaved w/ h1 scores
                    for k in range(0, nkt, 4):
                        for kk in range(k, min(k + 4, nkt)):
                            emit_score(j, 1, kk, pts)
                        if vq:
                            emit_v(j, vq.pop(0), xts)
                    while vq:
                        emit_v(j, vq.pop(0), xts)

                po0 = emit_pv(j, 0, nkt, pts)
                inv0 = emit_den(j, 0, nkt, pts)
                po1 = emit_pv(j, 1, nkt, pts)
                inv1 = emit_den(j, 1, nkt, pts)
                bcs0 = emit_bc(j, 0, inv0)
                if j > 0 and debug_stop is None:
                    emit_proj(j - 1, range(4 * (j - 1), 4 * (j - 1) + 2))
                emit_normmul(j, 0, po0, bcs0)
                bcs1 = emit_bc(j, 1, inv1)
                if j > 0 and debug_stop is None:
                    emit_proj(j - 1, range(4 * (j - 1) + 2, 4 * (j - 1) + 4))
                emit_normmul(j, 1, po1, bcs1)

            if debug_stop is None:
                emit_proj(nj - 1, range(4 * (nj - 1), 4 * (nj - 1) + 4))

            if debug_stop == "B":
                for f in range(4):
                    ob = outp.tile([128, t], bf16, tag="ob", name=f"obB{f}")
                    nc.vector.tensor_copy(ob[:], qk_sb[f][:])
                    nc.sync.dma_start(out=out_d[f * 128:(f + 1) * 128, 0:t],
                                      in_=ob[:])
                for g in range(ntt):
                    ob2 = outp.tile([128, 2 * D], bf16, tag="ob",
                                    name=f"obV{g}")
                    nc.vector.tensor_copy(ob2[:], v_sb[g][:])
                    nc.sync.dma_start(
                        out=out_d[(g % 4) * 128:(g % 4 + 1) * 128,
                                  t + (g // 4) * 256: t + (g // 4) * 256
                                  + 2 * D],
                        in_=ob2[:])
            if debug_stop == "C":
                for h in range(HPC):
                    ob = outp.tile([128, t], bf16, tag="ob", name=f"obC{h}")
                    nc.vector.tensor_copy(ob[:], attn_sb[h][:])
                    nc.sync.dma_start(out=out_d[h * 128:(h + 1) * 128, 0:t],
                                      in_=ob[:])

    nc.compile()
    return nc


# ---------------------------------------------------------------- host prep
def _rope_perm():
    p = np.empty(E, dtype=np.int64)
    for h in range(H):
        b = h * D
        p[b:b + 64] = b + np.arange(0, D, 2)
        p[b + 64:b + D] = b + np.arange(1, D, 2)
    return p


def _tables(t=T):
    # match reference: fp32 theta, fp32 angles
    theta = (1.0 / (BASE ** (np.arange(0, D, 2, dtype=np.float32) / np.float32(D)))
             ).astype(np.float32)
    m = np.arange(t, dtype=np.float32)
    fr = np.outer(m, theta).astype(np.float32)        # [t, 64]
    cos = np.cos(fr).T.astype(np.float32)             # [64, t]
    sin = np.sin(fr).T.astype(np.float32)
    cos2 = np.ascontiguousarray(np.concatenate([cos, cos], 0))
    sin2 = np.ascontiguousarray(np.concatenate([-sin, sin], 0))
    return cos2, sin2


def _masks():
    import ml_dtypes
    a = np.arange(128)[:, None]
    b = np.arange(QC)[None, :]
    return np.ascontiguousarray(
        np.stack([(b >= a + 128 * o) for o in range(4)])
        .astype(ml_dtypes.bfloat16))


def _prep_inputs(x, w_attn, w_proj, t=T):
    import ml_dtypes
    bf16 = ml_dtypes.bfloat16
    x2 = np.asarray(x, dtype=np.float32).reshape(t, E)
    # [j, half, partition, cc*QC] with per-partition contiguous runs
    xT = np.ascontiguousarray(
        x2.T.reshape(2, E // 256, 128, t // QC, QC)
        .transpose(3, 0, 2, 1, 4)
        .reshape(t // QC, 2, 128, (E // 256) * QC).astype(bf16))
    perm = _rope_perm()
    scale = np.float32(1.0) / np.sqrt(np.float32(D))
    wq = np.asarray(w_attn[0:E])[perm] * scale
    wk = np.asarray(w_attn[E:2 * E])[perm]
    wv = np.asarray(w_attn[2 * E:3 * E])
    import ml_dtypes as _mld
    cos2, sin2 = _tables(t)
    cos2 = cos2.astype(_mld.bfloat16)
    sin2 = sin2.astype(_mld.bfloat16)
    masks = _masks()
    in_maps = []
    for c in range(N_CORES):
        rows = slice(c * CL, (c + 1) * CL)
        wqkv = np.concatenate([wq[rows], wk[rows], wv[rows]], axis=0)  # [768, E]
        in_maps.append({
            "xT": xT,
            "wqkvT": np.ascontiguousarray(wqkv.T.astype(bf16)),
            "wprojT": np.ascontiguousarray(
                np.asarray(w_proj)[:, rows].T.astype(bf16)),
            "cos2": cos2,
            "sin2": sin2,
            "masks": masks,
        })
    return in_maps


# ---------------------------------------------------------------- cached runner
def _get_runner(t=T, debug_stop=None):
    """Build the Bass module once and return a cached jitted executor.

    Mirrors concourse.bass2jax.run_bass_via_pjrt's multi-core branch, but
    keeps the jitted callable so repeated kernel() calls don't recompile.
    """
    key = ("runner", t, debug_stop)
    if key in _CACHE:
        return _CACHE[key]
    import jax
    from concourse import bass2jax, mybir
    from jax.experimental.shard_map import shard_map
    from jax.sharding import Mesh, PartitionSpec

    nc = _build_nc(t, debug_stop)
    bass2jax.install_neuronx_cc_hook()

    partition_name = (nc.partition_id_tensor.name if nc.partition_id_tensor
                      else None)
    in_names, out_names, out_avals, zero_shapes = [], [], [], []
    for alloc in nc.m.functions[0].allocations:
        if not isinstance(alloc, mybir.MemoryLocationSet):
            continue
        name = alloc.memorylocations[0].name
        if alloc.kind == "ExternalInput":
            if name != partition_name:
                in_names.append(name)
        elif alloc.kind == "ExternalOutput":
            shape = tuple(alloc.tensor_shape)
            dtype = mybir.dt.np(alloc.dtype)
            out_names.append(name)
            out_avals.append(jax.core.ShapedArray(shape, dtype))
            zero_shapes.append((shape, dtype))
    n_params = len(in_names)
    all_in_names = list(in_names) + list(out_names)
    if partition_name is not None:
        all_in_names.append(partition_name)

    def _body(*args):
        operands = list(args)
        if partition_name is not None:
            operands.append(bass2jax.partition_id_tensor())
        outs = bass2jax._bass_exec_p.bind(
            *operands,
            out_avals=tuple(out_avals),
            in_names=tuple(all_in_names),
            out_names=tuple(out_names),
            lowering_input_output_aliases=(),
            sim_require_finite=True,
            sim_require_nnan=True,
            nc=nc,
        )
        return tuple(outs)

    devices = jax.devices()[:N_CORES]
    mesh = Mesh(np.asarray(devices), ("core",))
    donate = tuple(range(n_params, n_params + len(out_names)))
    sharded = jax.jit(
        shard_map(_body, mesh=mesh,
                  in_specs=(PartitionSpec("core"),) * (n_params + len(out_names)),
                  out_specs=(PartitionSpec("core"),) * len(out_names)),
        donate_argnums=donate, keep_unused=True)

    runner = {"fn": sharded, "in_names": in_names, "out_names": out_names,
              "out_avals": out_avals, "zero_shapes": zero_shapes, "nc": nc}
    _CACHE[key] = runner
    return runner


def _run(in_maps, t=T, debug_stop=None):
    r = _get_runner(t, debug_stop)
    concat_in = [
        np.concatenate([np.asarray(in_maps[c][name]) for c in range(N_CORES)],
                       axis=0)
        for name in r["in_names"]
    ]
    concat_zeros = [np.zeros((N_CORES * s[0], *s[1:]), d)
                    for (s, d) in r["zero_shapes"]]
    out_arrs = r["fn"](*concat_in, *concat_zeros)
    outs = []
    for c in range(N_CORES):
        outs.append({
            name: np.asarray(out_arrs[i]).reshape(N_CORES,
                                                  *r["out_avals"][i].shape)[c]
            for i, name in enumerate(r["out_names"])
        })
    return outs


# ---------------------------------------------------------------- entry point
def kernel(x, w_attn, w_proj):
    x = np.asarray(x, dtype=np.float32)
    w_attn = np.asarray(w_attn, dtype=np.float32)
    w_proj = np.asarray(w_proj, dtype=np.float32)
    in_maps = _prep_inputs(x, w_attn, w_proj)
    outs = _run(in_maps)
    acc = outs[0]["out"].astype(np.float32)
    for c in range(1, N_CORES):
        acc = acc + outs[c]["out"].astype(np.float32)
    return acc.reshape(B, T, E).astype(np.float32)


# revision 35
# speedup vs baseline: 1.0502x; 1.0502x over previous
"""Causal self-attention (B=1, T=2048, E=2048, 16 heads, RoPE) on 8 TRN2 NeuronCores.

Strategy: tensor-parallel over heads (2 heads/core). Each core computes
QKV for its heads, RoPE, causal softmax attention, and a PARTIAL output
projection over its 256 contraction columns of w_proj. The host sums the
8 partial [T, E] outputs (no on-device collectives).

All matmul operands are bfloat16 (fp32 PSUM accumulation): same PE rate
as f32r at these tile sizes, but half the DMA bytes, half the LDWEIGHTS
cycles, and 2-4x DVE throughput. End-to-end max-rel error ~4e-3.

Fused single-pass schedule per 512-query chunk j:
  QKV(j) matmuls + RoPE  |  scores S^T(j,h,k) -> exp -> P  |  PV + denom
  with proj(j-1) matmuls as PE filler while ACT drains exps.
Scores are computed transposed (S^T[kt, qt] = kT.T @ qT) so P^T feeds the
V matmul directly and out^T [d, qt] is exactly the lhsT the projection
needs. Causal masking: only lower-triangle kt-tiles computed; diagonal
tiles get multiplicative 0/1 masks after exp. Normalization deferred:
out^T = V^T P^T scaled by broadcast(1/rowsum), where the row sums come
from an elementwise tree-accumulation of the P tiles (DVE + gpsimd)
followed by a single ones-vector matmul, and 1/x = exp(-ln(x)) on ACT
(the DVE reciprocal instruction costs 3.3us).
"""
import sys

for _p in ("/opt/trn_rl_repo",):
    if _p not in sys.path:
        sys.path.append(_p)

import numpy as np

B, T, E = 1, 2048, 2048
H, D = 16, 128
N_CORES = 8
HPC = H // N_CORES          # heads per core
CL = HPC * D                # contraction columns per core (256)
QC = 512                    # qt chunk (PSUM bank width in fp32)
BASE = 10000.0

_CACHE: dict = {}


# ---------------------------------------------------------------- device build
def _build_nc(t=T, debug_stop=None):
    import concourse.tile as tile
    from concourse import bacc, mybir
    from contextlib import ExitStack

    f32 = mybir.dt.float32
    f32r = mybir.dt.float32r
    bf16 = mybir.dt.bfloat16
    nj = t // QC            # qt chunks
    ntt = t // 128          # t tiles
    nct = E // 128          # contraction tiles
    nhalf = nct // 2

    nc = bacc.Bacc("TRN2", target_bir_lowering=False, debug=False,
                   enable_asserts=False, num_devices=N_CORES)
    # x pre-transposed, per-partition contiguous: [j, half, 128, 8*QC]
    xT_d = nc.dram_tensor("xT", [t // QC, 2, 128, nhalf * QC], bf16,
                          kind="ExternalInput").ap()
    wqkvT_d = nc.dram_tensor("wqkvT", [E, 6 * 128], bf16,
                             kind="ExternalInput").ap()
    wprojT_d = nc.dram_tensor("wprojT", [CL, E], bf16, kind="ExternalInput").ap()
    cos2_d = nc.dram_tensor("cos2", [128, t], bf16, kind="ExternalInput").ap()
    sin2_d = nc.dram_tensor("sin2", [128, t], bf16, kind="ExternalInput").ap()
    masks_d = nc.dram_tensor("masks", [4, 128, QC], bf16,
                             kind="ExternalInput").ap()
    out_d = nc.dram_tensor("out", [t, E], bf16, kind="ExternalOutput").ap()

    Exp = mybir.ActivationFunctionType.Exp
    Ln = mybir.ActivationFunctionType.Ln

    with tile.TileContext(nc) as tc:
        with ExitStack() as per:  # persistent pools
            const = per.enter_context(tc.tile_pool(name="const", bufs=1))
            wpp = per.enter_context(tc.tile_pool(name="wpp", bufs=1))
            wqp = per.enter_context(tc.tile_pool(name="wqp", bufs=1))
            qkp = per.enter_context(tc.tile_pool(name="qkp", bufs=1))
            vp = per.enter_context(tc.tile_pool(name="vp", bufs=1))
            atp = per.enter_context(tc.tile_pool(name="atp", bufs=1))
            cstp = per.enter_context(tc.tile_pool(name="cstp", bufs=1))
            mkp = per.enter_context(tc.tile_pool(name="mkp", bufs=1))
            xtr = per.enter_context(tc.tile_pool(name="xtr", bufs=5))
            rtmp = per.enter_context(tc.tile_pool(name="rtmp", bufs=2))
            ptp = per.enter_context(tc.tile_pool(name="ptp", bufs=26))
            accp = per.enter_context(tc.tile_pool(name="accp", bufs=8))
            ctmp = per.enter_context(tc.tile_pool(name="ctmp", bufs=4))
            outp = per.enter_context(tc.tile_pool(name="outp", bufs=3))
            ps = per.enter_context(tc.tile_pool(name="ps", bufs=1, space="PSUM"))

            # ones vectors
            ones_row_f = const.tile([1, 128], f32)
            nc.vector.memset(ones_row_f[:], 1.0)
            ones_row = const.tile([1, 128], f32r)
            nc.vector.tensor_copy(ones_row[:], ones_row_f[:])
            ones_col_f = const.tile([128, 1], f32)
            nc.vector.memset(ones_col_f[:], 1.0)
            ones_col = const.tile([128, 1], bf16)
            nc.vector.tensor_copy(ones_col[:], ones_col_f[:])

            # persistent activations (bf16)
            qk_sb = [qkp.tile([128, t], bf16, tag=f"qk{f}", name=f"qk{f}")
                     for f in range(4)]
            v_sb = [vp.tile([128, 2 * D], bf16, tag=f"v{g}", name=f"v{g}")
                    for g in range(ntt)]
            attn_sb = [atp.tile([128, t], bf16, tag=f"at{h}", name=f"at{h}")
                       for h in range(HPC)]

            # ---------- prologue DMAs ----------
            # The vector sequencer boots earliest (~4us vs ~8-9 for
            # sync/scalar/gpsimd), so it carries the critical-path start:
            # first weight tiles, then chunk-0 x. Remaining weights on sync,
            # tables/masks on gpsimd, proj weights on scalar.
            WC = 6 * 128
            wq_all = wqp.tile([128, nct * WC], bf16)
            wsrc = wqkvT_d.rearrange("(c p) f -> p c f", c=nct)

            def dma_w(eng, c0, c1):
                dst = wq_all[:, c0 * WC:c1 * WC].rearrange(
                    "p (c f) -> p c f", c=c1 - c0)
                eng.dma_start(out=dst, in_=wsrc[:, c0:c1, :])

            def wq(c, lo, hi):
                return wq_all[:, c * WC + lo * 128:c * WC + hi * 128]

            cos2_sb = cstp.tile([128, t], bf16)
            sin2_sb = cstp.tile([128, t], bf16)
            mask_sb = mkp.tile([128, 4 * QC], bf16)
            wp_sb = []
            for hh in range(HPC):
                wp_sb.append(wpp.tile([128, E], bf16, tag=f"wp{hh}",
                                      name=f"wp{hh}"))

            def dma_x(j, eng=None, split=False):
                eng = eng or nc.sync
                xts = []
                for q in range(2):
                    xh = xtr.tile([128, nhalf * QC], bf16, tag="xt",
                                  name=f"xt{j}_{q}")
                    if split and q == 0:
                        eng.dma_start(out=xh[:, 0:4 * QC],
                                      in_=xT_d[j, q][:, 0:4 * QC])
                        eng.dma_start(out=xh[:, 4 * QC:],
                                      in_=xT_d[j, q][:, 4 * QC:])
                    else:
                        eng.dma_start(out=xh[:], in_=xT_d[j, q])
                    for cc in range(nhalf):
                        xts.append(xh[:, cc * QC:(cc + 1) * QC])
                return xts

            # chunk-0-critical stream in fine-grained, interleaved issues so
            # the global DMA-channel FIFOs deliver (w, x0) in consumption
            # order; x1 / proj weights / tables strictly behind it
            dma_w(nc.gpsimd, 0, 2)
            xh0 = xtr.tile([128, nhalf * QC], bf16, tag="xt", name="xt0_0")
            xh1 = xtr.tile([128, nhalf * QC], bf16, tag="xt", name="xt0_1")
            nc.gpsimd.dma_start(out=xh0[:, 0:4 * QC], in_=xT_d[0, 0][:, 0:4 * QC])
            dma_w(nc.scalar, 2, 5)
            nc.gpsimd.dma_start(out=xh0[:, 4 * QC:], in_=xT_d[0, 0][:, 4 * QC:])
            dma_w(nc.scalar, 5, 9)
            nc.gpsimd.dma_start(out=xh1[:, 0:4 * QC], in_=xT_d[0, 1][:, 0:4 * QC])
            dma_w(nc.sync, 9, 13)
            nc.gpsimd.dma_start(out=xh1[:, 4 * QC:], in_=xT_d[0, 1][:, 4 * QC:])
            dma_w(nc.sync, 13, 16)
            x_tiles = {0: [xh0[:, cc * QC:(cc + 1) * QC] for cc in range(nhalf)]
                       + [xh1[:, cc * QC:(cc + 1) * QC] for cc in range(nhalf)]}
            x_tiles[1] = dma_x(1, eng=nc.sync)
            for hh in range(HPC):
                nc.scalar.dma_start(out=wp_sb[hh][:],
                                    in_=wprojT_d[hh * 128:(hh + 1) * 128, :])
            nc.scalar.dma_start(out=cos2_sb[:], in_=cos2_d[:])
            nc.gpsimd.dma_start(out=sin2_sb[:], in_=sin2_d[:])
            nc.gpsimd.dma_start(
                out=mask_sb[:].rearrange("p (o b) -> p o b", o=4),
                in_=masks_d.rearrange("o p b -> p o b"))

            # ---------- fused chunk loop ----------
            def emit_rope(j, f, pq):
                """RoPE on DVE: out = pq*cos2 + swap_halves(pq)*sin2,
                with sin2 = [-sin; sin]; writes qk_sb[f] in bf16."""
                jsl = slice(j * QC, (j + 1) * QC)
                tA = rtmp.tile([128, QC], bf16, tag="tA", name=f"tA{j}_{f}")
                nc.vector.tensor_mul(tA[:], pq[:], cos2_sb[:, jsl])
                tB = rtmp.tile([128, QC], bf16, tag="tB", name=f"tB{j}_{f}")
                nc.vector.tensor_mul(tB[0:64, :], pq[64:128, :],
                                     sin2_sb[0:64, jsl])
                nc.vector.tensor_mul(tB[64:128, :], pq[0:64, :],
                                     sin2_sb[64:128, jsl])
                nc.vector.tensor_add(qk_sb[f][:, jsl], tA[:], tB[:])

            def emit_qkv_f(j, f, xts):
                """q/k tile f for chunk j: 16-matmul group + RoPE -> qk_sb."""
                pq = ps.tile([128, QC], f32, tag="q", bufs=3, name=f"pq{j}_{f}")
                for c in range(nct):
                    nc.tensor.matmul(pq[:], wq(c, f, f + 1), xts[c],
                                     start=(c == 0), stop=(c == nct - 1))
                emit_rope(j, f, pq)

            def emit_v(j, tt, xts):
                pv = ps.tile([128, 2 * D], f32, tag="q", bufs=3,
                             name=f"pv{j}_{tt}")
                for c in range(nct):
                    nc.tensor.matmul(pv[:], xts[c][:, tt * 128:(tt + 1) * 128],
                                     wq(c, 4, 6),
                                     start=(c == 0), stop=(c == nct - 1))
                if tt % 2 == 0:
                    nc.scalar.copy(v_sb[j * 4 + tt][:], pv[:])
                else:
                    nc.vector.tensor_copy(v_sb[j * 4 + tt][:], pv[:])

            def emit_score(j, h, k, pts, accs):
                """S^T tile (j,h,k): matmul -> exp (-> mask) -> P tile."""
                jsl = slice(j * QC, (j + 1) * QC)
                stp = ps.tile([128, QC], f32, tag="s", bufs=3,
                              name=f"st{j}_{h}_{k}")
                nc.tensor.matmul(stp[:], qk_sb[2 + h][:, k * 128:(k + 1) * 128],
                                 qk_sb[h][:, jsl], start=True, stop=True)
                pt = ptp.tile([128, QC], bf16, tag="pt", name=f"pt{j}_{h}_{k}")
                o = k - 4 * j
                if o >= 0:
                    e = ctmp.tile([128, QC], bf16, tag="e", name=f"e{j}_{h}_{k}")
                    nc.scalar.activation(e[:], stp[:], Exp)
                    nc.vector.tensor_mul(pt[:], e[:],
                                         mask_sb[:, o * QC:(o + 1) * QC])
                else:
                    nc.scalar.activation(pt[:], stp[:], Exp)
                pts[(h, k)] = pt
                # binary-counter tree accumulation of P tiles for the softmax
                # denominator: level-0 merges (paced by exp arrivals) on the
                # idle gpsimd engine, upper levels (latency-critical) on DVE.
                stack = accs[h]
                node = (0, pt)
                while stack and stack[-1][0] == node[0]:
                    lv, prev = stack.pop()
                    na = accp.tile([128, QC], bf16, tag="acc",
                                   name=f"acc{j}_{h}_{k}_{lv}")
                    eng = nc.gpsimd if lv == 0 else nc.vector
                    eng.tensor_add(na[:], prev[:], node[1][:])
                    node = (lv + 1, na)
                stack.append(node)

            def finish_acc(j, h, accs):
                """Collapse remaining tree levels into one [128, QC] tile."""
                stack = accs[h]
                node = stack.pop()
                while stack:
                    lv, prev = stack.pop()
                    na = accp.tile([128, QC], bf16, tag="acc",
                                   name=f"accf{j}_{h}_{lv}")
                    eng = nc.gpsimd if lv == 0 else nc.vector
                    eng.tensor_add(na[:], prev[:], node[1][:])
                    node = (lv + 1, na)
                return node[1]

            def emit_pv(j, h, nkt, pts):
                po = ps.tile([128, QC], f32, tag="o", bufs=2, name=f"po{j}_{h}")
                for k in range(nkt):
                    nc.tensor.matmul(po[:], v_sb[k][:, h * D:(h + 1) * D],
                                     pts[(h, k)][:], start=(k == 0),
                                     stop=(k == nkt - 1))
                return po

            def emit_den(j, h, acc):
                """denominator: single ones-matmul of the accumulated P sum,
                then a one-instruction DVE approximate reciprocal (~18 bits;
                the exact DVE reciprocal costs 3.3us, ACT Ln/Exp thrashes
                activation tables)."""
                ssp = ps.tile([1, QC], f32, tag="s", bufs=3,
                              name=f"ssp{j}_{h}")
                nc.tensor.matmul(ssp[:], ones_col[:], acc[:],
                                 start=True, stop=True)
                rcp = ctmp.tile([1, QC], f32, tag="rcp", bufs=2,
                                name=f"rcp{j}_{h}")
                nc.vector.reciprocal_approx_fast(out=rcp[:], in_=ssp[:])
                inv = ctmp.tile([1, QC], f32r, tag="inv", bufs=4,
                                name=f"inv{j}_{h}")
                with nc.allow_low_precision(reason="f32r softmax denominators"):
                    nc.vector.tensor_copy(inv[:], rcp[:])
                return inv

            def emit_bc(j, h, inv):
                """broadcast 1/sum across partitions via rank-1 matmul."""
                bc = ps.tile([128, QC], f32, tag="q", bufs=3, name=f"bc{j}_{h}")
                nc.tensor.matmul(bc[:], ones_row[:], inv[:], start=True,
                                 stop=True)
                bcs = ctmp.tile([128, QC], f32, tag="bcs", bufs=2,
                                name=f"bcs{j}_{h}")
                nc.scalar.copy(bcs[:], bc[:])
                return bcs

            def emit_normmul(j, h, po, bcs):
                jsl = slice(j * QC, (j + 1) * QC)
                nc.vector.tensor_mul(attn_sb[h][:, jsl], po[:], bcs[:])

            def emit_proj(jj, tts):
                """projection + output DMA for t-tiles tts of chunk jj."""
                for tt in tts:
                    ob = outp.tile([128, E], bf16, tag="ob", name=f"obD{tt}")
                    for oc in range(E // 512):
                        pp = ps.tile([128, 512], f32, tag="q", bufs=3,
                                     name=f"pp{tt}_{oc}")
                        for h in range(HPC):
                            nc.tensor.matmul(
                                pp[:], attn_sb[h][:, tt * 128:(tt + 1) * 128],
                                wp_sb[h][:, oc * 512:(oc + 1) * 512],
                                start=(h == 0), stop=(h == HPC - 1))
                        if oc % 2 == 0:
                            nc.vector.tensor_copy(
                                ob[:, oc * 512:(oc + 1) * 512], pp[:])
                        else:
                            nc.scalar.copy(
                                ob[:, oc * 512:(oc + 1) * 512], pp[:])
                    eng = nc.sync if tt % 2 == 0 else nc.scalar
                    eng.dma_start(out=out_d[tt * 128:(tt + 1) * 128, :],
                                  in_=ob[:])

            for j in range(nj):
                xts = x_tiles.pop(j)
                if j + 2 < nj:
                    x_tiles[j + 2] = dma_x(j + 2)
                nkt = 4 * (j + 1)
                pts: dict = {}
                accs: dict = {0: [], 1: []}

                if j == 0:
                    # c-major first chunk: 8 concurrent PSUM accumulation
                    # groups so compute paces the weight/x DMA arrival
                    # instead of waiting for the full contraction's worth.
                    qt = [ps.tile([128, QC], f32, tag="q" if f < 2 else "s",
                                  bufs=3, name=f"pq0_{f}") for f in range(4)]
                    vtags = ("q", "s", "o", "o")
                    vt = [ps.tile([128, 2 * D], f32, tag=vtags[tt],
                                  bufs=2 if vtags[tt] == "o" else 3,
                                  name=f"pv0_{tt}") for tt in range(4)]
                    for c in range(nct):
                        for f in range(4):
                            nc.tensor.matmul(qt[f][:], wq(c, f, f + 1), xts[c],
                                             start=(c == 0),
                                             stop=(c == nct - 1))
                        for tt in range(4):
                            nc.tensor.matmul(
                                vt[tt][:], xts[c][:, tt * 128:(tt + 1) * 128],
                                wq(c, 4, 6), start=(c == 0),
                                stop=(c == nct - 1))
                    for f in range(4):
                        emit_rope(0, f, qt[f])
                    for tt in range(4):
                        if tt % 2 == 0:
                            nc.scalar.copy(v_sb[tt][:], vt[tt][:])
                        else:
                            nc.vector.tensor_copy(v_sb[tt][:], vt[tt][:])
                    for k in range(nkt):
                        emit_score(j, 0, k, pts, accs)
                    for k in range(nkt):
                        emit_score(j, 1, k, pts, accs)
                else:
                    # QKV interleaved with h=0 scores (old k-tiles need only
                    # q(j,h0))
                    emit_qkv_f(j, 0, xts)
                    old = list(range(4 * j))   # k-tiles from earlier chunks
                    emit_qkv_f(j, 1, xts)
                    for k in old[0:4]:
                        emit_score(j, 0, k, pts, accs)
                    emit_qkv_f(j, 2, xts)
                    for k in old[4:8]:
                        emit_score(j, 0, k, pts, accs)
                    emit_qkv_f(j, 3, xts)
                    for k in old[8:12]:
                        emit_score(j, 0, k, pts, accs)
                    # diagonal tiles need k(j) (f=2,3 above)
                    for k in range(4 * j, nkt):
                        emit_score(j, 0, k, pts, accs)
                    emit_v(j, 0, xts)
                    emit_v(j, 1, xts)
                    vq = [2, 3]  # remaining v tiles, interleaved w/ h1 scores
                    for k in range(0, nkt, 4):
                        for kk in range(k, min(k + 4, nkt)):
                            emit_score(j, 1, kk, pts, accs)
                        if vq:
                            emit_v(j, vq.pop(0), xts)
                    while vq:
                        emit_v(j, vq.pop(0), xts)

                acc0 = finish_acc(j, 0, accs)
                acc1 = finish_acc(j, 1, accs)
                po0 = emit_pv(j, 0, nkt, pts)
                inv0 = emit_den(j, 0, acc0)
                po1 = emit_pv(j, 1, nkt, pts)
                inv1 = emit_den(j, 1, acc1)
                bcs0 = emit_bc(j, 0, inv0)
                if j > 0 and debug_stop is None:
                    emit_proj(j - 1, range(4 * (j - 1), 4 * (j - 1) + 2))
                emit_normmul(j, 0, po0, bcs0)
                bcs1 = emit_bc(j, 1, inv1)
                if j > 0 and debug_stop is None:
                    emit_proj(j - 1, range(4 * (j - 1) + 2, 4 * (j - 1) + 4))
                emit_normmul(j, 1, po1, bcs1)

            if debug_stop is None:
                emit_proj(nj - 1, range(4 * (nj - 1), 4 * (nj - 1) + 4))

            if debug_stop == "B":
                for f in range(4):
                    ob = outp.tile([128, t], bf16, tag="ob", name=f"obB{f}")
                    nc.vector.tensor_copy(ob[:], qk_sb[f][:])
                    nc.sync.dma_start(out=out_d[f * 128:(f + 1) * 128, 0:t],
                                      in_=ob[:])
                for g in range(ntt):
                    ob2 = outp.tile([128, 2 * D], bf16, tag="ob",
                                    name=f"obV{g}")
                    nc.vector.tensor_copy(ob2[:], v_sb[g][:])
                    nc.sync.dma_start(
                        out=out_d[(g % 4) * 128:(g % 4 + 1) * 128,
                                  t + (g // 4) * 256: t + (g // 4) * 256
                                  + 2 * D],
                        in_=ob2[:])
            if debug_stop == "C":
                for h in range(HPC):
                    ob = outp.tile([128, t], bf16, tag="ob", name=f"obC{h}")
                    nc.vector.tensor_copy(ob[:], attn_sb[h][:])
                    nc.sync.dma_start(out=out_d[h * 128:(h + 1) * 128, 0:t],
                                      in_=ob[:])

    nc.compile()
    return nc


# ---------------------------------------------------------------- host prep
def _rope_perm():
    p = np.empty(E, dtype=np.int64)
    for h in range(H):
        b = h * D
        p[b:b + 64] = b + np.arange(0, D, 2)
        p[b + 64:b + D] = b + np.arange(1, D, 2)
    return p


def _tables(t=T):
    # match reference: fp32 theta, fp32 angles
    theta = (1.0 / (BASE ** (np.arange(0, D, 2, dtype=np.float32) / np.float32(D)))
             ).astype(np.float32)
    m = np.arange(t, dtype=np.float32)
    fr = np.outer(m, theta).astype(np.float32)        # [t, 64]
    cos = np.cos(fr).T.astype(np.float32)             # [64, t]
    sin = np.sin(fr).T.astype(np.float32)
    cos2 = np.ascontiguousarray(np.concatenate([cos, cos], 0))
    sin2 = np.ascontiguousarray(np.concatenate([-sin, sin], 0))
    return cos2, sin2


def _masks():
    import ml_dtypes
    a = np.arange(128)[:, None]
    b = np.arange(QC)[None, :]
    return np.ascontiguousarray(
        np.stack([(b >= a + 128 * o) for o in range(4)])
        .astype(ml_dtypes.bfloat16))


def _prep_inputs(x, w_attn, w_proj, t=T):
    import ml_dtypes
    bf16 = ml_dtypes.bfloat16
    x2 = np.asarray(x, dtype=np.float32).reshape(t, E)
    # [j, half, partition, cc*QC] with per-partition contiguous runs
    xT = np.ascontiguousarray(
        x2.T.reshape(2, E // 256, 128, t // QC, QC)
        .transpose(3, 0, 2, 1, 4)
        .reshape(t // QC, 2, 128, (E // 256) * QC).astype(bf16))
    perm = _rope_perm()
    scale = np.float32(1.0) / np.sqrt(np.float32(D))
    wq = np.asarray(w_attn[0:E])[perm] * scale
    wk = np.asarray(w_attn[E:2 * E])[perm]
    wv = np.asarray(w_attn[2 * E:3 * E])
    import ml_dtypes as _mld
    cos2, sin2 = _tables(t)
    cos2 = cos2.astype(_mld.bfloat16)
    sin2 = sin2.astype(_mld.bfloat16)
    masks = _masks()
    in_maps = []
    for c in range(N_CORES):
        rows = slice(c * CL, (c + 1) * CL)
        wqkv = np.concatenate([wq[rows], wk[rows], wv[rows]], axis=0)  # [768, E]
        in_maps.append({
            "xT": xT,
            "wqkvT": np.ascontiguousarray(wqkv.T.astype(bf16)),
            "wprojT": np.ascontiguousarray(
                np.asarray(w_proj)[:, rows].T.astype(bf16)),
            "cos2": cos2,
            "sin2": sin2,
            "masks": masks,
        })
    return in_maps


# ---------------------------------------------------------------- cached runner
def _get_runner(t=T, debug_stop=None):
    """Build the Bass module once and return a cached jitted executor.

    Mirrors concourse.bass2jax.run_bass_via_pjrt's multi-core branch, but
    keeps the jitted callable so repeated kernel() calls don't recompile.
    """
    key = ("runner", t, debug_stop)
    if key in _CACHE:
        return _CACHE[key]
    import jax
    from concourse import bass2jax, mybir
    from jax.experimental.shard_map import shard_map
    from jax.sharding import Mesh, PartitionSpec

    nc = _build_nc(t, debug_stop)
    bass2jax.install_neuronx_cc_hook()

    partition_name = (nc.partition_id_tensor.name if nc.partition_id_tensor
                      else None)
    in_names, out_names, out_avals, zero_shapes = [], [], [], []
    for alloc in nc.m.functions[0].allocations:
        if not isinstance(alloc, mybir.MemoryLocationSet):
            continue
        name = alloc.memorylocations[0].name
        if alloc.kind == "ExternalInput":
            if name != partition_name:
                in_names.append(name)
        elif alloc.kind == "ExternalOutput":
            shape = tuple(alloc.tensor_shape)
            dtype = mybir.dt.np(alloc.dtype)
            out_names.append(name)
            out_avals.append(jax.core.ShapedArray(shape, dtype))
            zero_shapes.append((shape, dtype))
    n_params = len(in_names)
    all_in_names = list(in_names) + list(out_names)
    if partition_name is not None:
        all_in_names.append(partition_name)

    def _body(*args):
        operands = list(args)
        if partition_name is not None:
            operands.append(bass2jax.partition_id_tensor())
        outs = bass2jax._bass_exec_p.bind(
            *operands,
            out_avals=tuple(out_avals),
            in_names=tuple(all_in_names),
            out_names=tuple(out_names),
            lowering_input_output_aliases=(),
            sim_require_finite=True,
            sim_require_nnan=True,
            nc=nc,
        )
        return tuple(outs)

    devices = jax.devices()[:N_CORES]
    mesh = Mesh(np.asarray(devices), ("core",))
    donate = tuple(range(n_params, n_params + len(out_names)))
    sharded = jax.jit(
        shard_map(_body, mesh=mesh,
                  in_specs=(PartitionSpec("core"),) * (n_params + len(out_names)),
                  out_specs=(PartitionSpec("core"),) * len(out_names)),
        donate_argnums=donate, keep_unused=True)

    runner = {"fn": sharded, "in_names": in_names, "out_names": out_names,
              "out_avals": out_avals, "zero_shapes": zero_shapes, "nc": nc}
    _CACHE[key] = runner
    return runner


def _run(in_maps, t=T, debug_stop=None):
    r = _get_runner(t, debug_stop)
    concat_in = [
        np.concatenate([np.asarray(in_maps[c][name]) for c in range(N_CORES)],
                       axis=0)
        for name in r["in_names"]
    ]
    concat_zeros = [np.zeros((N_CORES * s[0], *s[1:]), d)
                    for (s, d) in r["zero_shapes"]]
    out_arrs = r["fn"](*concat_in, *concat_zeros)
    outs = []
    for c in range(N_CORES):
        outs.append({
            name: np.asarray(out_arrs[i]).reshape(N_CORES,
                                                  *r["out_avals"][i].shape)[c]
            for i, name in enumerate(r["out_names"])
        })
    return outs


# ---------------------------------------------------------------- entry point
def kernel(x, w_attn, w_proj):
    x = np.asarray(x, dtype=np.float32)
    w_attn = np.asarray(w_attn, dtype=np.float32)
    w_proj = np.asarray(w_proj, dtype=np.float32)
    in_maps = _prep_inputs(x, w_attn, w_proj)
    outs = _run(in_maps)
    acc = outs[0]["out"].astype(np.float32)
    for c in range(1, N_CORES):
        acc = acc + outs[c]["out"].astype(np.float32)
    return acc.reshape(B, T, E).astype(np.float32)
